# revision 1
# baseline (speedup 1.0000x reference)
"""Trainium2 Bass kernel for nn_Attention_50964081935360.

Single-query attention with a global-Frobenius-norm score scale:
  scores[b,s] = key[b,s,:] . query[b,:]
  denom      = ||key||_F  (over the WHOLE key tensor, all batches)
  p          = softmax(scores/denom) masked to s < seq_lens[b], renormalized
  out        = p[..., None] + 1e-15

Sharding: data-parallel over batch B=32 across 8 NeuronCores (4 batches per
core). Cross-core communication is a TWO-STAGE scalar AllReduce of the key
shard's sum of squares: AR1 covers super-tiles 0..14 and launches ~94% into
the DMA stream, absorbing the (large, variable) inter-core start skew under
the stream; AR2 covers just the last super-tile and costs only ~8 us of
mesh latency after the cores are aligned.

Per-core plan (memory-bound; key shard is 64 MiB, HBM floor ~187 us/core):
  - 15x 4 MiB super-tile DMAs on the sync HWDGE ring, one dma_start each,
    laid out [p, (j d)] with s = 1024g + 8p + j so each partition reads
    32 KiB CONTIGUOUS (~338 GB/s sustained); last super-tile in 4x 1 MiB
    chunks to shorten the post-stream drain.
  - DVE: affine_mul_reduce per s-tile column for scores; ACT: Square+accum
    chunks for the local ssq; TensorE all-ones matmuls do every partition
    reduce/broadcast (gpsimd cross-lane ops stall ~6 us on pool config).
  - AR1 (super-tiles 0..14) fires ~94% into the stream so its completion
    aligns the cores right before AR2 — placing it earlier makes AR2
    re-pay the start skew. AR2 then costs only ~8 us of mesh latency.
    Both cc_in DMAs ride the scalar ring behind their ACT producers (a
    sync-ring trigger's sem wait would stall key-load triggers).
  - inv = rsqrt(global ssq) via DVE bit-trick + 2 Newton steps; the Exp
    ACT table is preloaded during AR2, so zero table switches post-AR.
  - Epilogue fused across batches: one exp (per-partition scale), one
    masked multiply, one [p, b, t]-view column reduce, PE partition sum,
    one strided output DMA (no on-chip transposes).
  A warm-up AllReduce at kernel start pays the ncfw wakeup latency.
"""

import sys

import numpy as np

if "/opt/trn_rl_repo" not in sys.path:
    sys.path.insert(0, "/opt/trn_rl_repo")

import concourse.bacc as bacc
import concourse.bass as bass
import concourse.mybir as mybir
import concourse.tile as tile
from concourse.bass_isa import ReduceOp
from concourse.bass_utils import run_bass_kernel_spmd

B, S, D = 32, 4096, 1024
NCORES = 8
BPC = B // NCORES  # batches per core
P = 128            # s-tile partition size
NT = S // P        # s-tiles per batch (32)
NC_TILES = BPC * NT  # tiles per core (128)
PERTURB = 1e-15

F32 = mybir.dt.float32
I32 = mybir.dt.int32
ALU = mybir.AluOpType
ACTF = mybir.ActivationFunctionType

SUB = 8        # s-tiles per key super-tile
NG = NT // SUB  # super-tiles per batch (4)
KEY_BUFS = 4   # in-flight key super-tiles (4 MiB each)
NSQ = 4        # ACT square ops per super-tile (PSUM tile = 8 KiB/partition)


def build() -> bass.Bass:
    nc = bacc.Bacc(
        "TRN2", target_bir_lowering=False, debug=False, num_devices=NCORES
    )
    key_ext = nc.declare_dram_parameter("key", [BPC, S, D], F32, isOutput=False)
    q_ext = nc.declare_dram_parameter("query", [BPC, D], F32, isOutput=False)
    sl_ext = nc.declare_dram_parameter("seq_lens", [1, BPC], I32, isOutput=False)
    out_ext = nc.declare_dram_parameter("out", [BPC, S, 1], F32, isOutput=True)

    # Collective bounce buffers (internal DRAM; output must be Shared).
    # Two-stage ssq AllReduce: AR1 covers super-tiles 0..14 and launches
    # ~94% into the stream (absorbing inter-core skew under the stream);
    # AR2 covers only the last super-tile and pays just mesh latency.
    cc_in = nc.dram_tensor("cc_in", [1, 1], F32)
    cc_out = nc.dram_tensor("cc_out", [1, 1], F32, addr_space="Shared")
    cc_in2 = nc.dram_tensor("cc_in2", [1, 1], F32)
    cc_out2 = nc.dram_tensor("cc_out2", [1, 1], F32, addr_space="Shared")
    # Dummy collective buffers: a warm-up AllReduce at kernel start pays the
    # ncfw wakeup latency so the real one at the end doesn't.
    ccw_in = nc.dram_tensor("ccw_in", [1, 1], F32)
    ccw_out = nc.dram_tensor("ccw_out", [1, 1], F32, addr_space="Shared")

    key_ap = key_ext.ap()
    out_ap = out_ext.ap()

    with tile.TileContext(nc) as tc:
        with (
            tc.tile_pool(name="keys", bufs=KEY_BUFS) as kpool,
            tc.tile_pool(name="amr_scratch", bufs=4) as amrpool,
            tc.tile_pool(name="sq_psum", bufs=1, space="PSUM") as sqpool,
            tc.tile_pool(name="mm_psum", bufs=1, space="PSUM") as psmall,
            tc.tile_pool(name="persist", bufs=1) as pp,
        ):
            # all-ones stationaries for TensorE partition reduce/broadcast
            # (PE is otherwise idle; avoids slow gpsimd cross-lane ops)
            ones_full = pp.tile([P, P], F32)
            nc.vector.memset(ones_full[:, :], 1.0)
            ones_row = pp.tile([1, P], F32)
            nc.vector.memset(ones_row[:, :], 1.0)
            # ---- setup: query broadcast, seq_lens, s-index ----
            # q/seq_lens ride the ACT HWDGE ring so they don't queue behind
            # the 512 KiB key loads on the sync ring.
            # q DMAs go FIRST on the sync ring (HWDGE FIFO per ring), so they
            # land before the 512 KiB key-load flood; batch 0's broadcast
            # alone gates the first AMR.
            def load_supertile(b, g):
                # one 4 MiB DMA per super-tile with CONTIGUOUS 32 KiB per
                # partition: tile[p, j*D+d] = key[b, g*SUB*P + p*SUB + j, d],
                # i.e. column block j holds s = g*SUB*P + SUB*p + j. 128
                # sequential 32 KiB descriptors keep HBM near line rate.
                # All key loads ride nc.sync: HWDGE triggers on the scalar
                # ring would queue behind ACT squares and starve the stream.
                kt = kpool.tile([P, SUB * D], F32, tag="key")
                src = key_ap[
                    b, g * SUB * P : (g + 1) * SUB * P, :
                ].rearrange("(p j) d -> p j d", p=P)
                dst = kt[:, :].rearrange("p (j d) -> p j d", d=D)
                nc.sync.dma_start(out=dst, in_=src)
                return kt

            q_tiles = []
            for b in range(BPC):
                qr = pp.tile([P, D], F32, tag=f"qrep{b}")
                # scalar (ACT) ring: empty at kernel start, so these tiny
                # loads don't delay the first key DMA trigger on sync.
                nc.scalar.dma_start(
                    out=qr[0:1, :], in_=q_ext.ap()[b : b + 1, :]
                )
                q_tiles.append(qr)
            for b in range(BPC):
                nc.gpsimd.partition_broadcast(q_tiles[b][:, :], q_tiles[b][0:1, :])
            q_rep = [q_tiles[b][:, :] for b in range(BPC)]

            # warm-up collective (result unused)
            warm = pp.tile([1, 1], F32)
            nc.vector.memset(warm[:, :], 0.0)
            nc.scalar.dma_start(out=ccw_in.ap()[:, :], in_=warm[:, :])
            nc.gpsimd.collective_compute(
                "AllReduce",
                ALU.add,
                replica_groups=[list(range(NCORES))],
                ins=[ccw_in.ap().opt()],
                outs=[ccw_out.ap().opt()],
            )

            sl_i = pp.tile([1, BPC], I32)
            nc.scalar.dma_start(out=sl_i[:, :], in_=sl_ext.ap()[:, :])
            sl_f = pp.tile([P, BPC], F32)
            nc.vector.tensor_copy(out=sl_f[0:1, :], in_=sl_i[:, :])
            nc.gpsimd.partition_broadcast(sl_f[:, :], sl_f[0:1, :])

            # s_idx[p, c=(g,j)] = SUB*p + SUB*P*g + j  (sequence position of
            # scores[p, c] under the contiguous-per-partition key layout)
            s_idx_i = pp.tile([P, NT], I32)
            nc.gpsimd.iota(
                s_idx_i[:, :],
                pattern=[[SUB * P, NG], [1, SUB]],
                base=0,
                channel_multiplier=SUB,
            )
            s_idx = pp.tile([P, NT], F32)
            nc.vector.tensor_copy(out=s_idx[:, :], in_=s_idx_i[:, :])

            # masks depend only on s_idx/seq_lens: compute them up front so
            # the post-AllReduce tail is shorter
            masks_all = pp.tile([P, BPC * NT], F32)
            for b in range(BPC):
                nc.vector.tensor_scalar(
                    out=masks_all[:, b * NT : (b + 1) * NT],
                    in0=s_idx[:, :],
                    scalar1=sl_f[:, b : b + 1],
                    scalar2=None,
                    op0=ALU.is_lt,
                )

            # ---- main streaming loop over key super-tiles ----
            scores = pp.tile([P, NC_TILES], F32)
            ssqcols = pp.tile([P, NSQ * BPC * NG], F32)

            for b in range(BPC):
                for g in range(NG):
                    if b == BPC - 1 and g == NG - 1:
                        continue  # last super-tile handled below, split fine
                    kt = load_supertile(b, g)
                    # scores columns: sum_d key*q (one DVE pass per s-tile)
                    for j in range(SUB):
                        c = b * NT + g * SUB + j
                        amr = amrpool.tile([P, D], F32, tag="amr")
                        nc.vector.affine_mul_reduce(
                            out=amr[:, :],
                            accum_out=scores[:, c : c + 1],
                            in0=kt[:, j * D : (j + 1) * D],
                            in1=q_rep[b][:, :],
                            scale=1.0,
                            bias=0.0,
                        )
                    # global ssq only needs a total: square+accum over a chunk
                    # of the super-tile per ACT op (amortizes ACT overheads);
                    # out goes to PSUM (unused otherwise, saves SBUF)
                    for h in range(NSQ):
                        c2 = NSQ * (b * NG + g) + h
                        w = SUB * D // NSQ
                        sq = sqpool.tile([P, w], F32, tag="sq")
                        nc.scalar.activation(
                            out=sq[:, :],
                            in_=kt[:, h * w : (h + 1) * w],
                            func=ACTF.Square,
                            accum_out=ssqcols[:, c2 : c2 + 1],
                        )

            # Last super-tile in four 1 MiB chunks so the post-stream compute
            # drain is ~2 AMRs + 1 square instead of a full 4 MiB tile.
            bl, gl = BPC - 1, NG - 1
            full_last = key_ap[
                bl, gl * SUB * P : (gl + 1) * SUB * P, :
            ].rearrange("(p j) d -> p j d", p=P)
            ktcs = []
            for jj in range(4):
                ktc = kpool.tile([P, 2 * D], F32, tag="keyc")
                ktcs.append(ktc)
            for jj in range(4):
                # all four chunk DMAs issue on sync BEFORE the AR1 cc_in
                # DMA below them in this loop would otherwise... (they must
                # precede any sem-waiting trigger in the sync FIFO)
                nc.sync.dma_start(
                    out=ktcs[jj][:, :].rearrange("p (j d) -> p j d", d=D),
                    in_=full_last[:, 2 * jj : 2 * jj + 2, :],
                )
            # AR1 chain emitted BEFORE the last super-tile: Tile keeps
            # per-engine emission order, so this DVE reduce runs as soon as
            # super-tiles 0..14's squares land (~94% of stream) and AR1
            # absorbs the inter-core skew under the stream's tail.
            nsplit = NSQ * (BPC * NG - 1)  # cols of super-tiles 0..14
            ssq_r = pp.tile([P, 1], F32)
            nc.vector.tensor_reduce(
                out=ssq_r[:, :], in_=ssqcols[:, 0:nsplit],
                axis=mybir.AxisListType.XYZW, op=ALU.add,
            )
            ssqp = psmall.tile([P, 1], F32, tag="ssqp")
            nc.tensor.matmul(
                ssqp[:, :], ones_full[:, :], ssq_r[:, :], start=True, stop=True
            )
            ssq_sb = pp.tile([1, 1], F32)
            nc.scalar.copy(out=ssq_sb[:, :], in_=ssqp[0:1, :])
            nc.scalar.dma_start(out=cc_in.ap()[:, :], in_=ssq_sb[:, :])
            nc.gpsimd.collective_compute(
                "AllReduce",
                ALU.add,
                replica_groups=[list(range(NCORES))],
                ins=[cc_in.ap().opt()],
                outs=[cc_out.ap().opt()],
            )

            for jj in range(4):
                ktc = ktcs[jj]
                for jc in range(2):
                    c = bl * NT + gl * SUB + 2 * jj + jc
                    amr = amrpool.tile([P, D], F32, tag="amr")
                    nc.vector.affine_mul_reduce(
                        out=amr[:, :],
                        accum_out=scores[:, c : c + 1],
                        in0=ktc[:, jc * D : (jc + 1) * D],
                        in1=q_rep[bl][:, :],
                        scale=1.0,
                        bias=0.0,
                    )
                c2 = NSQ * (bl * NG + gl) + jj
                sq = sqpool.tile([P, 2 * D], F32, tag="sq")
                nc.scalar.activation(
                    out=sq[:, :],
                    in_=ktc[:, :],
                    func=ACTF.Square,
                    accum_out=ssqcols[:, c2 : c2 + 1],
                )

            # ---- local ssq reduction -> scalars, two-stage AllReduce ----
            # DVE free-dim reduce (fast) + TensorE all-ones matmul for the
            # partition reduce — avoids gpsimd CROSS_LANE_REDUCE's ~6 us
            # pool-config setup stall on the critical path.
            ssq_r2 = pp.tile([P, 1], F32)
            nc.vector.tensor_reduce(
                out=ssq_r2[:, :], in_=ssqcols[:, nsplit:],
                axis=mybir.AxisListType.XYZW, op=ALU.add,
            )
            ssqp2 = psmall.tile([P, 1], F32, tag="ssqp2")
            nc.tensor.matmul(
                ssqp2[:, :], ones_full[:, :], ssq_r2[:, :],
                start=True, stop=True,
            )
            ssq_sb2 = pp.tile([1, 1], F32)
            nc.scalar.copy(out=ssq_sb2[:, :], in_=ssqp2[0:1, :])
            nc.scalar.dma_start(out=cc_in2.ap()[:, :], in_=ssq_sb2[:, :])
            nc.gpsimd.collective_compute(
                "AllReduce",
                ALU.add,
                replica_groups=[list(range(NCORES))],
                ins=[cc_in2.ap().opt()],
                outs=[cc_out2.ap().opt()],
            )
            # Preload the Exp ACT table while AR2 is in flight. The inv
            # chain below is DVE-only, so Exp is the only table the tail
            # ever needs — no switches on the critical path.
            tdum = pp.tile([P, 1], F32)
            nc.scalar.activation(out=tdum[:, :], in_=ssqp[:, :], func=ACTF.Exp)

            # broadcast AR1's output to all partitions DURING AR2 (k=1 ones
            # matmul, start of a PSUM accumulation group), then accumulate
            # AR2's output in after it lands — the post-AR2 path only pays
            # one tiny matmul instead of DMA+reduce+matmul.
            ga_sb = pp.tile([1, 1], F32)
            nc.sync.dma_start(out=ga_sb[:, :], in_=cc_out.ap()[:, :])
            gb_sb = pp.tile([1, 1], F32)
            nc.sync.dma_start(out=gb_sb[:, :], in_=cc_out2.ap()[:, :])
            gbp = psmall.tile([P, 1], F32, tag="gbp")
            nc.tensor.matmul(
                gbp[:, :], ones_row[:, :], ga_sb[:, :], start=True, stop=False
            )
            nc.tensor.matmul(
                gbp[:, :], ones_row[:, :], gb_sb[:, :], start=False, stop=True
            )
            # inv = rsqrt(gssq) on DVE only: bit-trick seed + 2 Newton steps
            # (quadratic: ~3.4% -> 1.7e-3 -> 4e-6 rel err). Avoids Sqrt/Ln
            # ACT table loads after the AllReduce.
            g_sb = pp.tile([P, 1], F32)
            nc.vector.tensor_copy(out=g_sb[:, :], in_=gbp[:, :])
            magic = pp.tile([P, 1], I32)
            nc.vector.memset(magic[:, :], 0x5F3759DF)
            halfbits = pp.tile([P, 1], I32)
            nc.vector.tensor_scalar(
                out=halfbits[:, :],
                in0=g_sb[:, :].bitcast(I32),
                scalar1=1,
                scalar2=None,
                op0=ALU.logical_shift_right,
            )
            y_i = pp.tile([P, 1], I32)
            nc.vector.scalar_tensor_tensor(
                out=y_i[:, :],
                in0=magic[:, :],
                scalar=1,
                in1=halfbits[:, :],
                op0=ALU.mult,
                op1=ALU.subtract,
            )
            y = y_i[:, :].bitcast(F32)
            ya = pp.tile([P, 1], F32)
            yb = pp.tile([P, 1], F32)
            inv_rep = pp.tile([P, 1], F32)
            for it, (src, dst) in enumerate([(y, ya[:, :]), (ya[:, :], inv_rep[:, :])]):
                gy2 = pp.tile([P, 1], F32, tag=f"gy2_{it}")
                nc.vector.scalar_tensor_tensor(
                    out=gy2[:, :],
                    in0=src,
                    scalar=g_sb[:, 0:1],
                    in1=src,
                    op0=ALU.mult,
                    op1=ALU.mult,
                )
                corr = yb[:, :]
                nc.vector.tensor_scalar(
                    out=corr,
                    in0=gy2[:, :],
                    scalar1=-0.5,
                    scalar2=1.5,
                    op0=ALU.mult,
                    op1=ALU.add,
                )
                nc.vector.tensor_scalar(
                    out=dst,
                    in0=src,
                    scalar1=corr,
                    scalar2=None,
                    op0=ALU.mult,
                )

            # ---- epilogue: masked softmax, fused across batches ----
            # inv_rep is per-partition (same for every batch), so ONE exp
            # covers all 128 score columns; mask + per-batch column sums are
            # one multiply + one innermost-axis reduce over a [p, b, t] view.
            e_allt = pp.tile([P, NC_TILES], F32)
            nc.scalar.activation(
                out=e_allt[:, :],
                in_=scores[:, :],
                func=ACTF.Exp,
                scale=inv_rep[:, :],
            )
            em_allt = pp.tile([P, NC_TILES], F32)
            nc.vector.scalar_tensor_tensor(
                out=em_allt[:, :],
                in0=e_allt[:, :],
                scalar=1.0,
                in1=masks_all[:, :],
                op0=ALU.mult,
                op1=ALU.mult,
            )
            zcols = pp.tile([P, BPC], F32)
            nc.vector.tensor_reduce(
                out=zcols[:, :].rearrange("p (b o) -> p b o", o=1),
                in_=em_allt[:, :].rearrange("p (b t) -> p b t", t=NT),
                axis=mybir.AxisListType.X,
                op=ALU.add,
            )

            zsum = psmall.tile([P, BPC], F32, tag="zsum")
            nc.tensor.matmul(
                zsum[:, :], ones_full[:, :], zcols[:, :], start=True, stop=True
            )
            invz = pp.tile([P, BPC], F32)
            nc.vector.reciprocal(out=invz[:, :], in_=zsum[:, :])

            # s = SUB*P*g + SUB*p + j: partition p writes SUB contiguous
            # floats per (b, g) — no on-chip transpose, and ONE output DMA
            # for all batches (4 separate DMAs serialize ~800 ns apart on
            # the sync ring's FIFO).
            o_all = pp.tile([P, BPC * NT], F32)
            for b in range(BPC):
                nc.vector.tensor_scalar(
                    out=o_all[:, b * NT : (b + 1) * NT],
                    in0=em_allt[:, b * NT : (b + 1) * NT],
                    scalar1=invz[:, b : b + 1],
                    scalar2=PERTURB,
                    op0=ALU.mult,
                    op1=ALU.add,
                )
            dst = out_ap[:, :, 0].rearrange("b (g p j) -> p b g j", p=P, j=SUB)
            src = o_all[:, :].rearrange("p (b g j) -> p b g j", b=BPC, j=SUB)
            nc.sync.dma_start(out=dst, in_=src)

    nc.compile()
    return nc


_NC_CACHE = None


def _get_nc():
    global _NC_CACHE
    if _NC_CACHE is None:
        _NC_CACHE = build()
    return _NC_CACHE


def make_in_maps(key, query, seq_lens):
    key = np.ascontiguousarray(np.asarray(key, dtype=np.float32))
    query = np.ascontiguousarray(np.asarray(query, dtype=np.float32))
    seq_lens = np.ascontiguousarray(np.asarray(seq_lens, dtype=np.int32))
    in_maps = []
    for c in range(NCORES):
        lo, hi = c * BPC, (c + 1) * BPC
        in_maps.append(
            {
                "key": key[lo:hi],
                "query": query[lo:hi],
                "seq_lens": seq_lens[lo:hi].reshape(1, BPC),
            }
        )
    return in_maps


def kernel(key, query, seq_lens, **run_kwargs):
    nc = _get_nc()
    in_maps = make_in_maps(key, query, seq_lens)
    res = run_bass_kernel_spmd(
        nc, in_maps, core_ids=list(range(NCORES)), **run_kwargs
    )
    outs = [res.results[c]["out"].reshape(BPC, S, 1) for c in range(NCORES)]
    full = np.concatenate(outs, axis=0).astype(np.float32)
    if run_kwargs:
        kernel.last_results = res  # expose profile info to test harness
    return full



# revision 8
# speedup vs baseline: 1.0065x; 1.0065x over previous
"""Trainium2 Bass kernel for nn_Attention_50964081935360.

Single-query attention with a global-Frobenius-norm score scale:
  scores[b,s] = key[b,s,:] . query[b,:]
  denom      = ||key||_F  (over the WHOLE key tensor, all batches)
  p          = softmax(scores/denom) masked to s < seq_lens[b], renormalized
  out        = p[..., None] + 1e-15

Sharding: data-parallel over batch B=32 across 8 NeuronCores (4 batches per
core); one scalar AllReduce of the key-shard sum of squares, split in two
stages so the mesh latency hides under the stream.

v2 tail restructure (the v1 kernel lost ~45 us after the DMA stream ended):
  - The AR input reductions NEVER touch the DVE (which is saturated with
    affine_mul_reduce score columns until ~3 us after the stream ends).
    ssq column partial sums go ACT-Square+accum -> PE all-ones matmul
    (partition sum) -> ACT Copy+accum (column sum) -> cc_in DMA.
  - AR1 covers supertiles 0..13 and fires at ~77% of the stream; AR2
    covers 14..15 and fires ~2.5 us after the last key byte.
  - The exp epilogue runs on the AR1-only scale inv1 = rsqrt(g1) BEFORE
    AR2 lands; AR2's contribution is applied as an exact-to-fp32
    linearized correction E = E1*(1 + w*delta), delta = r*(-1/2 + 3r/8
    - 5r^2/16), r = g2/g1 (|w*delta| ~ 1e-4, cubic truncation ~1e-5 of
    inv -> ~1e-7 on p). Column sums of both E1*mask and E1*mask*w are
    pre-reduced, so the post-AR2 critical path is ~10 tiny DVE ops.
  - First and last supertiles stream in 4x 1 MiB chunks (subtile deps):
    the first AMR starts at ~10.5 us instead of ~22; the post-stream
    drain is one chunk (~3 us) instead of a full supertile.
  - masks/s_idx/q broadcast all sit in the DVE's pre-stream idle window.
Per-core: 22 key DMAs on the sync HWDGE ring, [p, (j d)] layout with
s = 1024g + 8p + j so each partition reads 32 KiB contiguous; DVE runs
128 AMR score columns (saturated ~98%); ACT runs one Square+accum per
supertile. A warm-up AllReduce (garbage input, result unused) pays the
ncfw wakeup before AR1 needs it.
"""

import sys

import numpy as np

if "/opt/trn_rl_repo" not in sys.path:
    sys.path.insert(0, "/opt/trn_rl_repo")

import concourse.bacc as bacc
import concourse.bass as bass
import concourse.mybir as mybir
import concourse.tile as tile
from concourse.bass_utils import run_bass_kernel_spmd

B, S, D = 32, 4096, 1024
NCORES = 8
BPC = B // NCORES  # batches per core
P = 128            # s-tile partition size
NT = S // P        # s-tiles per batch (32)
NC_TILES = BPC * NT  # score columns per core (128)
PERTURB = 1e-15

F32 = mybir.dt.float32
I32 = mybir.dt.int32
ALU = mybir.AluOpType
ACTF = mybir.ActivationFunctionType

SUB = 8          # s-tiles per key super-tile
NG = NT // SUB   # super-tiles per batch (4)
NST = BPC * NG   # super-tiles per core (16)
KEY_BUFS = 4
AR1_ST = 14      # super-tiles covered by AR1 (0..13); AR2 covers 14..15


def build() -> bass.Bass:
    nc = bacc.Bacc(
        "TRN2", target_bir_lowering=False, debug=False, num_devices=NCORES
    )
    key_ext = nc.declare_dram_parameter("key", [BPC, S, D], F32, isOutput=False)
    q_ext = nc.declare_dram_parameter("query", [BPC, D], F32, isOutput=False)
    sl_ext = nc.declare_dram_parameter("seq_lens", [1, BPC], I32, isOutput=False)
    out_ext = nc.declare_dram_parameter("out", [BPC, S, 1], F32, isOutput=True)

    cc_in = nc.dram_tensor("cc_in", [1, 1], F32)
    cc_out = nc.dram_tensor("cc_out", [1, 1], F32, addr_space="Shared")
    cc_in2 = nc.dram_tensor("cc_in2", [1, 1], F32)
    cc_out2 = nc.dram_tensor("cc_out2", [1, 1], F32, addr_space="Shared")
    ccw_in = nc.dram_tensor("ccw_in", [1, 1], F32)
    ccw_out = nc.dram_tensor("ccw_out", [1, 1], F32, addr_space="Shared")

    key_ap = key_ext.ap()
    out_ap = out_ext.ap()

    with tile.TileContext(nc) as tc:
        with (
            tc.tile_pool(name="keys", bufs=KEY_BUFS) as kpool,
            tc.tile_pool(name="amr_scratch", bufs=4) as amrpool,
            tc.tile_pool(name="mm_psum", bufs=1, space="PSUM") as psmall,
            tc.tile_pool(name="persist", bufs=1) as pp,
        ):
            # ---- persistent constants / small tiles ----
            ones_full = pp.tile([P, P], F32)
            nc.vector.memset(ones_full[:, :], 1.0)
            ones_row = pp.tile([1, P], F32)
            nc.vector.memset(ones_row[:, :], 1.0)

            # warm-up collective FIRST on gpsimd: input is garbage DRAM,
            # result unused; pays the ncfw wakeup latency (~50 us) long
            # before AR1 fires.
            nc.gpsimd.collective_compute(
                "AllReduce",
                ALU.add,
                replica_groups=[list(range(NCORES))],
                ins=[ccw_in.ap().opt()],
                outs=[ccw_out.ap().opt()],
            )

            # s_idx[p, c=(g,j)] = SUB*P*g + SUB*p + j (gpsimd iota, early)
            s_idx_i = pp.tile([P, NT], I32)
            nc.gpsimd.iota(
                s_idx_i[:, :],
                pattern=[[SUB * P, NG], [1, SUB]],
                base=0,
                channel_multiplier=SUB,
            )
            s_idx = pp.tile([P, NT], F32)
            nc.vector.tensor_copy(out=s_idx[:, :], in_=s_idx_i[:, :])

            # sl + q loads (scalar ring) and broadcasts (gpsimd); q_rep[0]
            # is ready before the first AMR needs it at ~10.5 us.
            sl_i = pp.tile([1, BPC], I32)
            nc.scalar.dma_start(out=sl_i[:, :], in_=sl_ext.ap()[:, :])
            q_tiles = []
            for b in range(BPC):
                qr = pp.tile([P, D], F32, tag=f"qrep{b}")
                nc.scalar.dma_start(out=qr[0:1, :], in_=q_ext.ap()[b : b + 1, :])
                q_tiles.append(qr)
            sl_f = pp.tile([P, BPC], F32)
            nc.vector.tensor_copy(out=sl_f[0:1, :], in_=sl_i[:, :])
            nc.gpsimd.partition_broadcast(q_tiles[0][:, :], q_tiles[0][0:1, :])
            nc.gpsimd.partition_broadcast(sl_f[:, :], sl_f[0:1, :])
            for b in range(1, BPC):
                nc.gpsimd.partition_broadcast(q_tiles[b][:, :], q_tiles[b][0:1, :])
            q_rep = [q_tiles[b][:, :] for b in range(BPC)]

            masks_all = pp.tile([P, NC_TILES], F32)
            for b in range(BPC):
                nc.vector.tensor_scalar(
                    out=masks_all[:, b * NT : (b + 1) * NT],
                    in0=s_idx[:, :],
                    scalar1=sl_f[:, b : b + 1],
                    scalar2=None,
                    op0=ALU.is_lt,
                )

            # ---- key streaming: 22 DMAs on the sync HWDGE ring ----
            # st0 and st15 go in 4x 1 MiB chunks INTO a normal key-tag
            # tile (subtile deps let per-chunk consumers start early).
            scores = pp.tile([P, NC_TILES], F32)
            ssq1 = pp.tile([P, AR1_ST + 3], F32)   # st0c0..3 + st1..13
            ssq2 = pp.tile([P, 5], F32)            # st14 + st15c0..3

            def st_src(t):
                b, g = divmod(t, NG)
                return key_ap[
                    b, g * SUB * P : (g + 1) * SUB * P, :
                ].rearrange("(p j) d -> p j d", p=P)

            key_tiles = [None] * NST

            def load_full(t):
                kt = kpool.tile([P, SUB * D], F32, tag="key")
                nc.sync.dma_start(
                    out=kt[:, :].rearrange("p (j d) -> p j d", d=D), in_=st_src(t)
                )
                key_tiles[t] = kt

            def load_chunked(t):
                kt = kpool.tile([P, SUB * D], F32, tag="key")
                src = st_src(t)
                for jj in range(4):
                    nc.sync.dma_start(
                        out=kt[:, jj * 2 * D : (jj + 1) * 2 * D].rearrange(
                            "p (j d) -> p j d", d=D
                        ),
                        in_=src[:, 2 * jj : 2 * jj + 2, :],
                    )
                key_tiles[t] = kt

            def amr_cols(t, js):
                b = t // NG
                kt = key_tiles[t]
                for j in js:
                    c = b * NT + (t % NG) * SUB + j
                    amr = amrpool.tile([P, D], F32, tag="amr")
                    nc.vector.affine_mul_reduce(
                        out=amr[:, :],
                        accum_out=scores[:, c : c + 1],
                        in0=kt[:, j * D : (j + 1) * D],
                        in1=q_rep[b][:, :],
                        scale=1.0,
                        bias=0.0,
                    )

            BF16 = mybir.dt.bfloat16

            def sqdump(w):
                # write-only dump for ACT Square (only accum_out matters)
                sqd = amrpool.tile(
                    [P, SUB * D], BF16, tag="sqd", bufs=1, name="sqd"
                )
                return sqd[:, 0:w]

            def sq_full(t, acc):
                nc.scalar.activation(
                    out=sqdump(SUB * D),
                    in_=key_tiles[t][:, :],
                    func=ACTF.Square,
                    accum_out=acc,
                )

            # --- emission: DMAs + compute interleaved per supertile ---
            load_chunked(0)
            for jj in range(4):
                nc.scalar.activation(
                    out=sqdump(2 * D),
                    in_=key_tiles[0][:, jj * 2 * D : (jj + 1) * 2 * D],
                    func=ACTF.Square,
                    accum_out=ssq1[:, jj : jj + 1],
                )
            amr_cols(0, range(SUB))

            for t in range(1, AR1_ST):
                load_full(t)
                sq_full(t, ssq1[:, 3 + t : 4 + t])
                amr_cols(t, range(SUB))

            # AR1 input: PE partition-sum -> ACT column-sum -> DMA.
            # No DVE involvement; fires as soon as sq13 lands (~77% of
            # the stream), absorbing inter-core skew under the stream.
            ps1 = psmall.tile([P, AR1_ST + 3], F32, tag="ps1")
            nc.tensor.matmul(
                ps1[:, :], ones_full[:, :], ssq1[:, :], start=True, stop=True
            )
            g1_sb = pp.tile([1, 1], F32)
            csd1 = pp.tile([1, AR1_ST + 3], F32, tag="csd1", name="csd1")
            nc.scalar.activation(
                out=csd1[:, :],
                in_=ps1[0:1, :],
                func=ACTF.Copy,
                accum_out=g1_sb[:, :],
            )
            nc.scalar.dma_start(out=cc_in.ap()[:, :], in_=g1_sb[:, :])
            nc.gpsimd.collective_compute(
                "AllReduce",
                ALU.add,
                replica_groups=[list(range(NCORES))],
                ins=[cc_in.ap().opt()],
                outs=[cc_out.ap().opt()],
            )

            # supertile 14 (full) + 15 (chunked)
            load_full(AR1_ST)
            sq_full(AR1_ST, ssq2[:, 0:1])
            amr_cols(AR1_ST, range(SUB))

            load_chunked(NST - 1)
            for jj in range(4):
                nc.scalar.activation(
                    out=sqdump(2 * D),
                    in_=key_tiles[NST - 1][:, jj * 2 * D : (jj + 1) * 2 * D],
                    func=ACTF.Square,
                    accum_out=ssq2[:, 1 + jj : 2 + jj],
                )
            amr_cols(NST - 1, range(SUB))

            # AR2 input: same PE+ACT path, ready ~2.5 us after last byte.
            ps2 = psmall.tile([P, 5], F32, tag="ps2")
            nc.tensor.matmul(
                ps2[:, :], ones_full[:, :], ssq2[:, :], start=True, stop=True
            )
            g2_sb = pp.tile([1, 1], F32)
            csd2 = pp.tile([1, 5], F32, tag="csd2", name="csd2")
            nc.scalar.activation(
                out=csd2[:, :],
                in_=ps2[0:1, :],
                func=ACTF.Copy,
                accum_out=g2_sb[:, :],
            )
            nc.scalar.dma_start(out=cc_in2.ap()[:, :], in_=g2_sb[:, :])
            nc.gpsimd.collective_compute(
                "AllReduce",
                ALU.add,
                replica_groups=[list(range(NCORES))],
                ins=[cc_in2.ap().opt()],
                outs=[cc_out2.ap().opt()],
            )

            # ---- pre-AR2 epilogue on the AR1-only scale ----
            # g1 arrives mid-stream; sqrt on ACT (default table), recip on
            # DVE. E1 = exp(scores * inv1) as soon as the last AMR lands.
            g1a_sb = pp.tile([1, 1], F32)
            nc.sync.dma_start(out=g1a_sb[:, :], in_=cc_out.ap()[:, :])
            g1b = psmall.tile([P, 1], F32, tag="g1b")
            nc.tensor.matmul(
                g1b[:, :], ones_row[:, :], g1a_sb[:, :], start=True, stop=True
            )
            g1r = pp.tile([P, 1], F32)
            nc.vector.tensor_copy(out=g1r[:, :], in_=g1b[:, :])
            g1sq = pp.tile([P, 1], F32)
            nc.scalar.activation(out=g1sq[:, :], in_=g1r[:, :], func=ACTF.Sqrt)
            inv1 = pp.tile([P, 1], F32)
            nc.vector.reciprocal(out=inv1[:, :], in_=g1sq[:, :])
            invg1 = pp.tile([P, 1], F32)  # 1/g1 = inv1^2, for r = g2/g1
            nc.vector.tensor_scalar(
                out=invg1[:, :], in0=inv1[:, :], scalar1=inv1[:, 0:1],
                scalar2=None, op0=ALU.mult,
            )

            e1 = pp.tile([P, NC_TILES], F32)
            nc.scalar.activation(
                out=e1[:, :], in_=scores[:, :], func=ACTF.Exp, scale=inv1[:, :]
            )
            w_t = pp.tile([P, NC_TILES], F32)
            nc.vector.tensor_scalar(
                out=w_t[:, :], in0=scores[:, :], scalar1=inv1[:, 0:1],
                scalar2=None, op0=ALU.mult,
            )
            em = pp.tile([P, NC_TILES], F32)
            nc.vector.tensor_tensor(
                out=em[:, :], in0=e1[:, :], in1=masks_all[:, :], op=ALU.mult
            )
            fm = pp.tile([P, NC_TILES], F32)
            nc.vector.tensor_tensor(
                out=fm[:, :], in0=em[:, :], in1=w_t[:, :], op=ALU.mult
            )
            zc = pp.tile([P, 2 * BPC], F32)  # [ze(4) | zf(4)] column sums
            nc.vector.tensor_reduce(
                out=zc[:, 0:BPC].rearrange("p (b o) -> p b o", o=1),
                in_=em[:, :].rearrange("p (b t) -> p b t", t=NT),
                axis=mybir.AxisListType.X,
                op=ALU.add,
            )
            nc.vector.tensor_reduce(
                out=zc[:, BPC : 2 * BPC].rearrange("p (b o) -> p b o", o=1),
                in_=fm[:, :].rearrange("p (b t) -> p b t", t=NT),
                axis=mybir.AxisListType.X,
                op=ALU.add,
            )
            zs = psmall.tile([P, 2 * BPC], F32, tag="zs")
            nc.tensor.matmul(
                zs[:, :], ones_full[:, :], zc[:, :], start=True, stop=True
            )
            zs_sb = pp.tile([P, 2 * BPC], F32)
            nc.vector.tensor_copy(out=zs_sb[:, :], in_=zs[:, :])

            # ---- post-AR2: linearized correction, ~10 tiny ops ----
            g2a_sb = pp.tile([1, 1], F32)
            nc.sync.dma_start(out=g2a_sb[:, :], in_=cc_out2.ap()[:, :])
            g2b = psmall.tile([P, 1], F32, tag="g2b")
            nc.tensor.matmul(
                g2b[:, :], ones_row[:, :], g2a_sb[:, :], start=True, stop=True
            )
            # r = g2/g1 ; delta = r*(-1/2 + r*(3/8 - (5/16) r))
            r_t = pp.tile([P, 1], F32)
            nc.vector.tensor_scalar(
                out=r_t[:, :], in0=g2b[:, :], scalar1=invg1[:, 0:1],
                scalar2=None, op0=ALU.mult,
            )
            h_t = pp.tile([P, 1], F32)
            nc.vector.tensor_scalar(
                out=h_t[:, :], in0=r_t[:, :], scalar1=-0.3125, scalar2=0.375,
                op0=ALU.mult, op1=ALU.add,
            )
            h2_t = pp.tile([P, 1], F32)  # h2 = h*r - 0.5
            nc.vector.tensor_scalar(
                out=h2_t[:, :], in0=h_t[:, :], scalar1=r_t[:, 0:1],
                scalar2=-0.5, op0=ALU.mult, op1=ALU.add,
            )
            delta = pp.tile([P, 1], F32)  # delta = h2 * r
            nc.vector.tensor_scalar(
                out=delta[:, :], in0=h2_t[:, :], scalar1=r_t[:, 0:1],
                scalar2=None, op0=ALU.mult,
            )
            # Z = ze + delta*zf ; invz = 1/Z
            z_t = pp.tile([P, BPC], F32)
            nc.vector.scalar_tensor_tensor(
                out=z_t[:, :],
                in0=zs_sb[:, BPC : 2 * BPC],
                scalar=delta[:, 0:1],
                in1=zs_sb[:, 0:BPC],
                op0=ALU.mult,
                op1=ALU.add,
            )
            invz = pp.tile([P, BPC], F32)
            nc.vector.reciprocal(out=invz[:, :], in_=z_t[:, :])
            # u = em + delta*fm ; o = u*invz_b + PERTURB
            u_t = pp.tile([P, NC_TILES], F32)
            nc.vector.scalar_tensor_tensor(
                out=u_t[:, :],
                in0=fm[:, :],
                scalar=delta[:, 0:1],
                in1=em[:, :],
                op0=ALU.mult,
                op1=ALU.add,
            )
            o_all = pp.tile([P, NC_TILES], F32)
            for b in range(BPC):
                nc.vector.tensor_scalar(
                    out=o_all[:, b * NT : (b + 1) * NT],
                    in0=u_t[:, b * NT : (b + 1) * NT],
                    scalar1=invz[:, b : b + 1],
                    scalar2=PERTURB,
                    op0=ALU.mult,
                    op1=ALU.add,
                )
            dst = out_ap[:, :, 0].rearrange("b (g p j) -> p b g j", p=P, j=SUB)
            src = o_all[:, :].rearrange("p (b g j) -> p b g j", b=BPC, j=SUB)
            nc.sync.dma_start(out=dst, in_=src)

    nc.compile()
    return nc


_NC_CACHE = None


def _get_nc():
    global _NC_CACHE
    if _NC_CACHE is None:
        _NC_CACHE = build()
    return _NC_CACHE


def make_in_maps(key, query, seq_lens):
    key = np.ascontiguousarray(np.asarray(key, dtype=np.float32))
    query = np.ascontiguousarray(np.asarray(query, dtype=np.float32))
    seq_lens = np.ascontiguousarray(np.asarray(seq_lens, dtype=np.int32))
    in_maps = []
    for c in range(NCORES):
        lo, hi = c * BPC, (c + 1) * BPC
        in_maps.append(
            {
                "key": key[lo:hi],
                "query": query[lo:hi],
                "seq_lens": seq_lens[lo:hi].reshape(1, BPC),
            }
        )
    return in_maps


def kernel(key, query, seq_lens, **run_kwargs):
    nc = _get_nc()
    in_maps = make_in_maps(key, query, seq_lens)
    res = run_bass_kernel_spmd(
        nc, in_maps, core_ids=list(range(NCORES)), **run_kwargs
    )
    outs = [res.results[c]["out"].reshape(BPC, S, 1) for c in range(NCORES)]
    full = np.concatenate(outs, axis=0).astype(np.float32)
    if run_kwargs:
        kernel.last_results = res  # expose profile info to test harness
    return full


# revision 10
# speedup vs baseline: 1.0310x; 1.0243x over previous
"""Trainium2 Bass kernel for nn_Attention_50964081935360.

Single-query attention with a global-Frobenius-norm score scale:
  scores[b,s] = key[b,s,:] . query[b,:]
  denom      = ||key||_F  (over the WHOLE key tensor, all batches)
  p          = softmax(scores/denom) masked to s < seq_lens[b], renormalized
  out        = p[..., None] + 1e-15

Sharding: data-parallel over batch B=32 across 8 NeuronCores (4 batches per
core); one scalar AllReduce of the key-shard sum of squares, split in two
stages so the mesh latency hides under the stream.

v2 tail restructure (the v1 kernel lost ~45 us after the DMA stream ended):
  - The AR input reductions NEVER touch the DVE (which is saturated with
    affine_mul_reduce score columns until ~3 us after the stream ends).
    ssq column partial sums go ACT-Square+accum -> PE all-ones matmul
    (partition sum) -> ACT Copy+accum (column sum) -> cc_in DMA.
  - AR1 covers supertiles 0..13 and fires at ~77% of the stream; AR2
    covers 14..15 and fires ~2.5 us after the last key byte.
  - The exp epilogue runs on the AR1-only scale inv1 = rsqrt(g1) BEFORE
    AR2 lands; AR2's contribution is applied as an exact-to-fp32
    linearized correction E = E1*(1 + w*delta), delta = r*(-1/2 + 3r/8
    - 5r^2/16), r = g2/g1 (|w*delta| ~ 1e-4, cubic truncation ~1e-5 of
    inv -> ~1e-7 on p). Column sums of both E1*mask and E1*mask*w are
    pre-reduced, so the post-AR2 critical path is ~10 tiny DVE ops.
  - First and last supertiles stream in 4x 1 MiB chunks (subtile deps):
    the first AMR starts at ~10.5 us instead of ~22; the post-stream
    drain is one chunk (~3 us) instead of a full supertile.
  - masks/s_idx/q broadcast all sit in the DVE's pre-stream idle window.
Per-core: 22 key DMAs on the sync HWDGE ring, [p, (j d)] layout with
s = 1024g + 8p + j so each partition reads 32 KiB contiguous; DVE runs
128 AMR score columns (saturated ~98%); ACT runs one Square+accum per
supertile. A warm-up AllReduce (garbage input, result unused) pays the
ncfw wakeup before AR1 needs it.
"""

import sys

import numpy as np

if "/opt/trn_rl_repo" not in sys.path:
    sys.path.insert(0, "/opt/trn_rl_repo")

import concourse.bacc as bacc
import concourse.bass as bass
import concourse.mybir as mybir
import concourse.tile as tile
from concourse.bass_utils import run_bass_kernel_spmd

B, S, D = 32, 4096, 1024
NCORES = 8
BPC = B // NCORES  # batches per core
P = 128            # s-tile partition size
NT = S // P        # s-tiles per batch (32)
NC_TILES = BPC * NT  # score columns per core (128)
PERTURB = 1e-15

F32 = mybir.dt.float32
I32 = mybir.dt.int32
ALU = mybir.AluOpType
ACTF = mybir.ActivationFunctionType

SUB = 8          # s-tiles per key super-tile
NG = NT // SUB   # super-tiles per batch (4)
NST = BPC * NG   # super-tiles per core (16)
KEY_BUFS = 4
AR1_ST = 14      # super-tiles covered by AR1 (0..13); AR2 covers 14..15


def build() -> bass.Bass:
    nc = bacc.Bacc(
        "TRN2", target_bir_lowering=False, debug=False, num_devices=NCORES
    )
    key_ext = nc.declare_dram_parameter("key", [BPC, S, D], F32, isOutput=False)
    q_ext = nc.declare_dram_parameter("query", [BPC, D], F32, isOutput=False)
    sl_ext = nc.declare_dram_parameter("seq_lens", [1, BPC], I32, isOutput=False)
    out_ext = nc.declare_dram_parameter("out", [BPC, S, 1], F32, isOutput=True)

    cc_in = nc.dram_tensor("cc_in", [1, 1], F32)
    cc_out = nc.dram_tensor("cc_out", [1, 1], F32, addr_space="Shared")
    cc_in2 = nc.dram_tensor("cc_in2", [1, 1], F32)
    cc_out2 = nc.dram_tensor("cc_out2", [1, 1], F32, addr_space="Shared")
    ccw_in = nc.dram_tensor("ccw_in", [1, 1], F32)
    ccw_out = nc.dram_tensor("ccw_out", [1, 1], F32, addr_space="Shared")

    key_ap = key_ext.ap()
    out_ap = out_ext.ap()

    with tile.TileContext(nc) as tc:
        with (
            tc.tile_pool(name="keys", bufs=KEY_BUFS) as kpool,
            tc.tile_pool(name="amr_scratch", bufs=4) as amrpool,
            tc.tile_pool(name="mm_psum", bufs=1, space="PSUM") as psmall,
            tc.tile_pool(name="persist", bufs=1) as pp,
        ):
            # ---- persistent constants / small tiles ----
            ones_full = pp.tile([P, P], F32)
            nc.vector.memset(ones_full[:, :], 1.0)
            ones_row = pp.tile([1, P], F32)
            nc.vector.memset(ones_row[:, :], 1.0)

            # warm-up collective FIRST on gpsimd: input is garbage DRAM,
            # result unused; pays the ncfw wakeup latency (~50 us) long
            # before AR1 fires.
            nc.gpsimd.collective_compute(
                "AllReduce",
                ALU.add,
                replica_groups=[list(range(NCORES))],
                ins=[ccw_in.ap().opt()],
                outs=[ccw_out.ap().opt()],
            )

            # s_idx[p, c=(g,j)] = SUB*P*g + SUB*p + j (gpsimd iota, early)
            s_idx_i = pp.tile([P, NT], I32)
            nc.gpsimd.iota(
                s_idx_i[:, :],
                pattern=[[SUB * P, NG], [1, SUB]],
                base=0,
                channel_multiplier=SUB,
            )
            s_idx = pp.tile([P, NT], F32)
            nc.vector.tensor_copy(out=s_idx[:, :], in_=s_idx_i[:, :])

            # sl + q loads (scalar ring) and broadcasts (gpsimd); q_rep[0]
            # is ready before the first AMR needs it at ~10.5 us.
            sl_i = pp.tile([1, BPC], I32)
            nc.scalar.dma_start(out=sl_i[:, :], in_=sl_ext.ap()[:, :])
            q_tiles = []
            for b in range(BPC):
                qr = pp.tile([P, D], F32, tag=f"qrep{b}")
                nc.scalar.dma_start(out=qr[0:1, :], in_=q_ext.ap()[b : b + 1, :])
                q_tiles.append(qr)
            sl_f = pp.tile([P, BPC], F32)
            nc.vector.tensor_copy(out=sl_f[0:1, :], in_=sl_i[:, :])
            nc.gpsimd.partition_broadcast(q_tiles[0][:, :], q_tiles[0][0:1, :])
            nc.gpsimd.partition_broadcast(sl_f[:, :], sl_f[0:1, :])
            for b in range(1, BPC):
                nc.gpsimd.partition_broadcast(q_tiles[b][:, :], q_tiles[b][0:1, :])
            q_rep = [q_tiles[b][:, :] for b in range(BPC)]

            masks_all = pp.tile([P, NC_TILES], F32)
            for b in range(BPC):
                nc.vector.tensor_scalar(
                    out=masks_all[:, b * NT : (b + 1) * NT],
                    in0=s_idx[:, :],
                    scalar1=sl_f[:, b : b + 1],
                    scalar2=None,
                    op0=ALU.is_lt,
                )

            # ---- key streaming: 22 DMAs on the sync HWDGE ring ----
            # st0 and st15 go in 4x 1 MiB chunks INTO a normal key-tag
            # tile (subtile deps let per-chunk consumers start early).
            scores = pp.tile([P, NC_TILES], F32)
            ssq1 = pp.tile([P, 4 * AR1_ST], F32)   # 4 chunk cols per st 0..13
            ssq2 = pp.tile([P, 9], F32)            # st14 x4 + st15 x5

            def st_src(t):
                b, g = divmod(t, NG)
                return key_ap[
                    b, g * SUB * P : (g + 1) * SUB * P, :
                ].rearrange("(p j) d -> p j d", p=P)

            key_tiles = [None] * NST

            def load_chunked(t, jsplits=((0, 2), (2, 4), (4, 6), (6, 8))):
                kt = kpool.tile([P, SUB * D], F32, tag="key")
                src = st_src(t)
                for j0, j1 in jsplits:
                    nc.sync.dma_start(
                        out=kt[:, j0 * D : j1 * D].rearrange(
                            "p (j d) -> p j d", d=D
                        ),
                        in_=src[:, j0:j1, :],
                    )
                key_tiles[t] = kt

            def amr_cols(t, js):
                b = t // NG
                kt = key_tiles[t]
                for j in js:
                    c = b * NT + (t % NG) * SUB + j
                    amr = amrpool.tile([P, D], F32, tag="amr")
                    nc.vector.affine_mul_reduce(
                        out=amr[:, :],
                        accum_out=scores[:, c : c + 1],
                        in0=kt[:, j * D : (j + 1) * D],
                        in1=q_rep[b][:, :],
                        scale=1.0,
                        bias=0.0,
                    )

            BF16 = mybir.dt.bfloat16

            def sqdump(w):
                # write-only dump for ACT Square (only accum_out matters)
                sqd = amrpool.tile(
                    [P, 2 * D], BF16, tag="sqd", bufs=1, name="sqd"
                )
                return sqd[:, 0:w]

            def sq_chunks(t, acc_tile, acc0, jsplits=((0, 2), (2, 4), (4, 6), (6, 8))):
                # one Square+accum per loaded chunk (pipelines with arrival)
                for i, (j0, j1) in enumerate(jsplits):
                    nc.scalar.activation(
                        out=sqdump((j1 - j0) * D),
                        in_=key_tiles[t][:, j0 * D : j1 * D],
                        func=ACTF.Square,
                        accum_out=acc_tile[:, acc0 + i : acc0 + i + 1],
                    )

            # --- emission: DMAs + compute interleaved per supertile ---
            for t in range(AR1_ST):
                load_chunked(t)
                sq_chunks(t, ssq1, 4 * t)
                amr_cols(t, range(SUB))

            # AR1 input: PE partition-sum -> ACT column-sum -> DMA.
            # No DVE involvement; fires as soon as sq13 lands (~77% of
            # the stream), absorbing inter-core skew under the stream.
            ps1 = psmall.tile([P, 4 * AR1_ST], F32, tag="ps1")
            nc.tensor.matmul(
                ps1[:, :], ones_full[:, :], ssq1[:, :], start=True, stop=True
            )
            g1_sb = pp.tile([1, 1], F32)
            csd1 = pp.tile([1, 4 * AR1_ST], F32, tag="csd1", name="csd1")
            nc.scalar.activation(
                out=csd1[:, :],
                in_=ps1[0:1, :],
                func=ACTF.Copy,
                accum_out=g1_sb[:, :],
            )
            nc.scalar.dma_start(out=cc_in.ap()[:, :], in_=g1_sb[:, :])
            nc.gpsimd.collective_compute(
                "AllReduce",
                ALU.add,
                replica_groups=[list(range(NCORES))],
                ins=[cc_in.ap().opt()],
                outs=[cc_out.ap().opt()],
            )

            # supertiles 14, 15 (AR2); last chunk is 512 KB so its square
            # (the AR2-input gate) finishes ~1.4 us after the last byte
            load_chunked(AR1_ST)
            sq_chunks(AR1_ST, ssq2, 0)
            amr_cols(AR1_ST, range(SUB))

            LASTSPLIT = ((0, 2), (2, 4), (4, 6), (6, 7), (7, 8))
            load_chunked(NST - 1, LASTSPLIT)
            sq_chunks(NST - 1, ssq2, 4, LASTSPLIT)
            amr_cols(NST - 1, range(SUB))

            # AR2 input: same PE+ACT path, ready ~2.5 us after last byte.
            ps2 = psmall.tile([P, 9], F32, tag="ps2")
            nc.tensor.matmul(
                ps2[:, :], ones_full[:, :], ssq2[:, :], start=True, stop=True
            )
            g2_sb = pp.tile([1, 1], F32)
            csd2 = pp.tile([1, 9], F32, tag="csd2", name="csd2")
            nc.scalar.activation(
                out=csd2[:, :],
                in_=ps2[0:1, :],
                func=ACTF.Copy,
                accum_out=g2_sb[:, :],
            )
            nc.scalar.dma_start(out=cc_in2.ap()[:, :], in_=g2_sb[:, :])
            nc.gpsimd.collective_compute(
                "AllReduce",
                ALU.add,
                replica_groups=[list(range(NCORES))],
                ins=[cc_in2.ap().opt()],
                outs=[cc_out2.ap().opt()],
            )

            # ---- pre-AR2 epilogue on the AR1-only scale ----
            # g1 arrives mid-stream; sqrt on ACT (default table), recip on
            # DVE. E1 = exp(scores * inv1) as soon as the last AMR lands.
            g1a_sb = pp.tile([1, 1], F32)
            nc.sync.dma_start(out=g1a_sb[:, :], in_=cc_out.ap()[:, :])
            g1b = psmall.tile([P, 1], F32, tag="g1b")
            nc.tensor.matmul(
                g1b[:, :], ones_row[:, :], g1a_sb[:, :], start=True, stop=True
            )
            # inv1 = rsqrt(g1): bit-trick seed + 2 Newton steps, DVE-only
            # (no ACT table loads on the tail path)
            g1r = pp.tile([P, 1], F32)
            nc.vector.tensor_copy(out=g1r[:, :], in_=g1b[:, :])
            magic = pp.tile([P, 1], I32)
            nc.vector.memset(magic[:, :], 0x5F3759DF)
            halfbits = pp.tile([P, 1], I32)
            nc.vector.tensor_scalar(
                out=halfbits[:, :], in0=g1r[:, :].bitcast(I32), scalar1=1,
                scalar2=None, op0=ALU.logical_shift_right,
            )
            y_i = pp.tile([P, 1], I32)
            nc.vector.scalar_tensor_tensor(
                out=y_i[:, :], in0=magic[:, :], scalar=1, in1=halfbits[:, :],
                op0=ALU.mult, op1=ALU.subtract,
            )
            y = y_i[:, :].bitcast(F32)
            ya = pp.tile([P, 1], F32)
            yb = pp.tile([P, 1], F32)
            inv1 = pp.tile([P, 1], F32)
            for it, (src_ap, dst) in enumerate(
                [(y, ya[:, :]), (ya[:, :], inv1[:, :])]
            ):
                gy2 = pp.tile([P, 1], F32, tag=f"gy2_{it}", name=f"gy2_{it}")
                nc.vector.scalar_tensor_tensor(
                    out=gy2[:, :], in0=src_ap, scalar=g1r[:, 0:1], in1=src_ap,
                    op0=ALU.mult, op1=ALU.mult,
                )
                nc.vector.tensor_scalar(
                    out=yb[:, :], in0=gy2[:, :], scalar1=-0.5, scalar2=1.5,
                    op0=ALU.mult, op1=ALU.add,
                )
                nc.vector.tensor_scalar(
                    out=dst, in0=src_ap, scalar1=yb[:, 0:1], scalar2=None,
                    op0=ALU.mult,
                )
            invg1 = pp.tile([P, 1], F32)  # 1/g1 = inv1^2, for r = g2/g1
            nc.vector.tensor_scalar(
                out=invg1[:, :], in0=inv1[:, :], scalar1=inv1[:, 0:1],
                scalar2=None, op0=ALU.mult,
            )

            e1 = pp.tile([P, NC_TILES], F32)
            nc.scalar.activation(
                out=e1[:, :], in_=scores[:, :], func=ACTF.Exp, scale=inv1[:, :]
            )
            w_t = pp.tile([P, NC_TILES], F32)
            nc.vector.tensor_scalar(
                out=w_t[:, :], in0=scores[:, :], scalar1=inv1[:, 0:1],
                scalar2=None, op0=ALU.mult,
            )
            em = pp.tile([P, NC_TILES], F32)
            nc.vector.tensor_tensor(
                out=em[:, :], in0=e1[:, :], in1=masks_all[:, :], op=ALU.mult
            )
            fm = pp.tile([P, NC_TILES], F32)
            nc.vector.tensor_tensor(
                out=fm[:, :], in0=em[:, :], in1=w_t[:, :], op=ALU.mult
            )
            zc = pp.tile([P, 2 * BPC], F32)  # [ze(4) | zf(4)] column sums
            nc.vector.tensor_reduce(
                out=zc[:, 0:BPC].rearrange("p (b o) -> p b o", o=1),
                in_=em[:, :].rearrange("p (b t) -> p b t", t=NT),
                axis=mybir.AxisListType.X,
                op=ALU.add,
            )
            nc.vector.tensor_reduce(
                out=zc[:, BPC : 2 * BPC].rearrange("p (b o) -> p b o", o=1),
                in_=fm[:, :].rearrange("p (b t) -> p b t", t=NT),
                axis=mybir.AxisListType.X,
                op=ALU.add,
            )
            zs = psmall.tile([P, 2 * BPC], F32, tag="zs")
            nc.tensor.matmul(
                zs[:, :], ones_full[:, :], zc[:, :], start=True, stop=True
            )
            zs_sb = pp.tile([P, 2 * BPC], F32)
            nc.vector.tensor_copy(out=zs_sb[:, :], in_=zs[:, :])

            # ---- post-AR2: linearized correction, ~10 tiny ops ----
            g2a_sb = pp.tile([1, 1], F32)
            nc.sync.dma_start(out=g2a_sb[:, :], in_=cc_out2.ap()[:, :])
            g2b = psmall.tile([P, 1], F32, tag="g2b")
            nc.tensor.matmul(
                g2b[:, :], ones_row[:, :], g2a_sb[:, :], start=True, stop=True
            )
            # r = g2/g1 ; delta = r*(-1/2 + r*(3/8 - (5/16) r))
            r_t = pp.tile([P, 1], F32)
            nc.vector.tensor_scalar(
                out=r_t[:, :], in0=g2b[:, :], scalar1=invg1[:, 0:1],
                scalar2=None, op0=ALU.mult,
            )
            h_t = pp.tile([P, 1], F32)  # h = 0.375*r - 0.5
            nc.vector.tensor_scalar(
                out=h_t[:, :], in0=r_t[:, :], scalar1=0.375, scalar2=-0.5,
                op0=ALU.mult, op1=ALU.add,
            )
            delta = pp.tile([P, 1], F32)  # delta = h * r
            nc.vector.tensor_scalar(
                out=delta[:, :], in0=h_t[:, :], scalar1=r_t[:, 0:1],
                scalar2=None, op0=ALU.mult,
            )
            # Z = ze + delta*zf ; invz = 1/Z
            z_t = pp.tile([P, BPC], F32)
            nc.vector.scalar_tensor_tensor(
                out=z_t[:, :],
                in0=zs_sb[:, BPC : 2 * BPC],
                scalar=delta[:, 0:1],
                in1=zs_sb[:, 0:BPC],
                op0=ALU.mult,
                op1=ALU.add,
            )
            invz = pp.tile([P, BPC], F32)
            nc.vector.reciprocal(out=invz[:, :], in_=z_t[:, :])
            # u = em + delta*fm ; o = u*invz_b + PERTURB
            u_t = pp.tile([P, NC_TILES], F32)
            nc.vector.scalar_tensor_tensor(
                out=u_t[:, :],
                in0=fm[:, :],
                scalar=delta[:, 0:1],
                in1=em[:, :],
                op0=ALU.mult,
                op1=ALU.add,
            )
            o_all = pp.tile([P, NC_TILES], F32)
            for b in range(BPC):
                nc.vector.tensor_scalar(
                    out=o_all[:, b * NT : (b + 1) * NT],
                    in0=u_t[:, b * NT : (b + 1) * NT],
                    scalar1=invz[:, b : b + 1],
                    scalar2=PERTURB,
                    op0=ALU.mult,
                    op1=ALU.add,
                )
            dst = out_ap[:, :, 0].rearrange("b (g p j) -> p b g j", p=P, j=SUB)
            src = o_all[:, :].rearrange("p (b g j) -> p b g j", b=BPC, j=SUB)
            nc.sync.dma_start(out=dst, in_=src)

    nc.compile()
    return nc


_NC_CACHE = None


def _get_nc():
    global _NC_CACHE
    if _NC_CACHE is None:
        _NC_CACHE = build()
    return _NC_CACHE


def make_in_maps(key, query, seq_lens):
    key = np.ascontiguousarray(np.asarray(key, dtype=np.float32))
    query = np.ascontiguousarray(np.asarray(query, dtype=np.float32))
    seq_lens = np.ascontiguousarray(np.asarray(seq_lens, dtype=np.int32))
    in_maps = []
    for c in range(NCORES):
        lo, hi = c * BPC, (c + 1) * BPC
        in_maps.append(
            {
                "key": key[lo:hi],
                "query": query[lo:hi],
                "seq_lens": seq_lens[lo:hi].reshape(1, BPC),
            }
        )
    return in_maps


def kernel(key, query, seq_lens, **run_kwargs):
    nc = _get_nc()
    in_maps = make_in_maps(key, query, seq_lens)
    res = run_bass_kernel_spmd(
        nc, in_maps, core_ids=list(range(NCORES)), **run_kwargs
    )
    outs = [res.results[c]["out"].reshape(BPC, S, 1) for c in range(NCORES)]
    full = np.concatenate(outs, axis=0).astype(np.float32)
    if run_kwargs:
        kernel.last_results = res  # expose profile info to test harness
    return full


# revision 11
# speedup vs baseline: 1.0470x; 1.0155x over previous
"""Trainium2 Bass kernel for nn_Attention_50964081935360.

Single-query attention with a global-Frobenius-norm score scale:
  scores[b,s] = key[b,s,:] . query[b,:]
  denom      = ||key||_F  (over the WHOLE key tensor, all batches)
  p          = softmax(scores/denom) masked to s < seq_lens[b], renormalized
  out        = p[..., None] + 1e-15

Sharding: data-parallel over batch B=32 across 8 NeuronCores (4 batches per
core); one scalar AllReduce of the key-shard sum of squares, split in two
stages so the mesh latency hides under the stream.

v2 tail restructure (the v1 kernel lost ~45 us after the DMA stream ended):
  - The AR input reductions NEVER touch the DVE (which is saturated with
    affine_mul_reduce score columns until ~3 us after the stream ends).
    ssq column partial sums go ACT-Square+accum -> PE all-ones matmul
    (partition sum) -> ACT Copy+accum (column sum) -> cc_in DMA.
  - AR1 covers supertiles 0..13 and fires at ~77% of the stream; AR2
    covers 14..15 and fires ~2.5 us after the last key byte.
  - The exp epilogue runs on the AR1-only scale inv1 = rsqrt(g1) BEFORE
    AR2 lands; AR2's contribution is applied as an exact-to-fp32
    linearized correction E = E1*(1 + w*delta), delta = r*(-1/2 + 3r/8
    - 5r^2/16), r = g2/g1 (|w*delta| ~ 1e-4, cubic truncation ~1e-5 of
    inv -> ~1e-7 on p). Column sums of both E1*mask and E1*mask*w are
    pre-reduced, so the post-AR2 critical path is ~10 tiny DVE ops.
  - First and last supertiles stream in 4x 1 MiB chunks (subtile deps):
    the first AMR starts at ~10.5 us instead of ~22; the post-stream
    drain is one chunk (~3 us) instead of a full supertile.
  - masks/s_idx/q broadcast all sit in the DVE's pre-stream idle window.
Per-core: 22 key DMAs on the sync HWDGE ring, [p, (j d)] layout with
s = 1024g + 8p + j so each partition reads 32 KiB contiguous; DVE runs
128 AMR score columns (saturated ~98%); ACT runs one Square+accum per
supertile. A warm-up AllReduce (garbage input, result unused) pays the
ncfw wakeup before AR1 needs it.
"""

import sys

import numpy as np

if "/opt/trn_rl_repo" not in sys.path:
    sys.path.insert(0, "/opt/trn_rl_repo")

import concourse.bacc as bacc
import concourse.bass as bass
import concourse.mybir as mybir
import concourse.tile as tile
from concourse.bass_utils import run_bass_kernel_spmd

B, S, D = 32, 4096, 1024
NCORES = 8
BPC = B // NCORES  # batches per core
P = 128            # s-tile partition size
NT = S // P        # s-tiles per batch (32)
NC_TILES = BPC * NT  # score columns per core (128)
PERTURB = 1e-15

F32 = mybir.dt.float32
I32 = mybir.dt.int32
ALU = mybir.AluOpType
ACTF = mybir.ActivationFunctionType

SUB = 8          # s-tiles per key super-tile
NG = NT // SUB   # super-tiles per batch (4)
NST = BPC * NG   # super-tiles per core (16)
KEY_BUFS = 4
AR1_ST = 14      # super-tiles covered by AR1 (0..13); AR2 covers 14..15


def build() -> bass.Bass:
    nc = bacc.Bacc(
        "TRN2", target_bir_lowering=False, debug=False, num_devices=NCORES
    )
    key_ext = nc.declare_dram_parameter("key", [BPC, S, D], F32, isOutput=False)
    q_ext = nc.declare_dram_parameter("query", [BPC, D], F32, isOutput=False)
    sl_ext = nc.declare_dram_parameter("seq_lens", [1, BPC], I32, isOutput=False)
    out_ext = nc.declare_dram_parameter("out", [BPC, S, 1], F32, isOutput=True)

    cc_in = nc.dram_tensor("cc_in", [1, 1], F32)
    cc_out = nc.dram_tensor("cc_out", [1, 1], F32, addr_space="Shared")
    cc_in2 = nc.dram_tensor("cc_in2", [1, 1], F32)
    cc_out2 = nc.dram_tensor("cc_out2", [1, 1], F32, addr_space="Shared")
    ccw_in = nc.dram_tensor("ccw_in", [1, 1], F32)
    ccw_out = nc.dram_tensor("ccw_out", [1, 1], F32, addr_space="Shared")

    key_ap = key_ext.ap()
    out_ap = out_ext.ap()

    with tile.TileContext(nc) as tc:
        with (
            tc.tile_pool(name="keys", bufs=KEY_BUFS) as kpool,
            tc.tile_pool(name="amr_scratch", bufs=4) as amrpool,
            tc.tile_pool(name="mm_psum", bufs=1, space="PSUM") as psmall,
            tc.tile_pool(name="persist", bufs=1) as pp,
        ):
            # ---- persistent constants / small tiles ----
            ones_full = pp.tile([P, P], F32)
            nc.vector.memset(ones_full[:, :], 1.0)
            ones_row = pp.tile([1, P], F32)
            nc.vector.memset(ones_row[:, :], 1.0)

            # warm-up collective FIRST on gpsimd: input is garbage DRAM,
            # result unused; pays the ncfw wakeup latency (~50 us) long
            # before AR1 fires.
            nc.gpsimd.collective_compute(
                "AllReduce",
                ALU.add,
                replica_groups=[list(range(NCORES))],
                ins=[ccw_in.ap().opt()],
                outs=[ccw_out.ap().opt()],
            )

            # s_idx[p, c=(g,j)] = SUB*P*g + SUB*p + j (gpsimd iota, early)
            s_idx_i = pp.tile([P, NT], I32)
            nc.gpsimd.iota(
                s_idx_i[:, :],
                pattern=[[SUB * P, NG], [1, SUB]],
                base=0,
                channel_multiplier=SUB,
            )
            s_idx = pp.tile([P, NT], F32)
            nc.vector.tensor_copy(out=s_idx[:, :], in_=s_idx_i[:, :])

            # sl + q loads (scalar ring) and broadcasts (gpsimd); q_rep[0]
            # is ready before the first AMR needs it at ~10.5 us.
            sl_i = pp.tile([1, BPC], I32)
            nc.scalar.dma_start(out=sl_i[:, :], in_=sl_ext.ap()[:, :])
            q_tiles = []
            for b in range(BPC):
                qr = pp.tile([P, D], F32, tag=f"qrep{b}")
                nc.scalar.dma_start(out=qr[0:1, :], in_=q_ext.ap()[b : b + 1, :])
                q_tiles.append(qr)
            sl_f = pp.tile([P, BPC], F32)
            nc.vector.tensor_copy(out=sl_f[0:1, :], in_=sl_i[:, :])
            nc.gpsimd.partition_broadcast(q_tiles[0][:, :], q_tiles[0][0:1, :])
            nc.gpsimd.partition_broadcast(sl_f[:, :], sl_f[0:1, :])
            for b in range(1, BPC):
                nc.gpsimd.partition_broadcast(q_tiles[b][:, :], q_tiles[b][0:1, :])
            q_rep = [q_tiles[b][:, :] for b in range(BPC)]

            masks_all = pp.tile([P, NC_TILES], F32)
            for b in range(BPC):
                nc.vector.tensor_scalar(
                    out=masks_all[:, b * NT : (b + 1) * NT],
                    in0=s_idx[:, :],
                    scalar1=sl_f[:, b : b + 1],
                    scalar2=None,
                    op0=ALU.is_lt,
                )

            # ---- key streaming: 22 DMAs on the sync HWDGE ring ----
            # st0 and st15 go in 4x 1 MiB chunks INTO a normal key-tag
            # tile (subtile deps let per-chunk consumers start early).
            scores = pp.tile([P, NC_TILES], F32)
            # ssq1 cols: st0 x4 chunks, st1..12 x1 each, st13 x4 chunks = 20
            ssq1 = pp.tile([P, 20], F32)
            ssq2 = pp.tile([P, 9], F32)            # st14 x4 + st15 x5

            def st_src(t):
                b, g = divmod(t, NG)
                return key_ap[
                    b, g * SUB * P : (g + 1) * SUB * P, :
                ].rearrange("(p j) d -> p j d", p=P)

            key_tiles = [None] * NST

            def load_chunked(t, jsplits=((0, 2), (2, 4), (4, 6), (6, 8))):
                kt = kpool.tile([P, SUB * D], F32, tag="key")
                src = st_src(t)
                for j0, j1 in jsplits:
                    nc.sync.dma_start(
                        out=kt[:, j0 * D : j1 * D].rearrange(
                            "p (j d) -> p j d", d=D
                        ),
                        in_=src[:, j0:j1, :],
                    )
                key_tiles[t] = kt

            def load_full(t):
                kt = kpool.tile([P, SUB * D], F32, tag="key")
                nc.sync.dma_start(
                    out=kt[:, :].rearrange("p (j d) -> p j d", d=D),
                    in_=st_src(t),
                )
                key_tiles[t] = kt

            def amr_cols(t, js):
                b = t // NG
                kt = key_tiles[t]
                for j in js:
                    c = b * NT + (t % NG) * SUB + j
                    amr = amrpool.tile([P, D], F32, tag="amr")
                    nc.vector.affine_mul_reduce(
                        out=amr[:, :],
                        accum_out=scores[:, c : c + 1],
                        in0=kt[:, j * D : (j + 1) * D],
                        in1=q_rep[b][:, :],
                        scale=1.0,
                        bias=0.0,
                    )

            BF16 = mybir.dt.bfloat16

            def sqdump(w):
                # write-only dump for ACT Square (only accum_out matters)
                sqd = amrpool.tile(
                    [P, SUB * D], BF16, tag="sqd", bufs=1, name="sqd"
                )
                return sqd[:, 0:w]

            def sq_chunks(t, acc_tile, acc0, jsplits=((0, 2), (2, 4), (4, 6), (6, 8))):
                # one Square+accum per loaded chunk (pipelines with arrival)
                for i, (j0, j1) in enumerate(jsplits):
                    nc.scalar.activation(
                        out=sqdump((j1 - j0) * D),
                        in_=key_tiles[t][:, j0 * D : j1 * D],
                        func=ACTF.Square,
                        accum_out=acc_tile[:, acc0 + i : acc0 + i + 1],
                    )

            # --- emission: DMAs + compute interleaved per supertile ---
            # st0 chunked (early DVE start), st1..12 full 4 MiB (best DMA
            # rate; their FD=8192 squares trail arrival by ~7 us, not on
            # any critical path), st13 chunked (AR1's gate fires early).
            load_chunked(0)
            sq_chunks(0, ssq1, 0)
            amr_cols(0, range(SUB))
            for t in range(1, AR1_ST - 1):
                load_full(t)
                nc.scalar.activation(
                    out=sqdump(SUB * D),
                    in_=key_tiles[t][:, :],
                    func=ACTF.Square,
                    accum_out=ssq1[:, 3 + t : 4 + t],
                )
                amr_cols(t, range(SUB))
            load_chunked(AR1_ST - 1)
            sq_chunks(AR1_ST - 1, ssq1, 16)
            amr_cols(AR1_ST - 1, range(SUB))

            # AR1 input: PE partition-sum -> ACT column-sum -> DMA.
            # No DVE involvement; fires as soon as sq13 lands (~77% of
            # the stream), absorbing inter-core skew under the stream.
            ps1 = psmall.tile([P, 20], F32, tag="ps1")
            nc.tensor.matmul(
                ps1[:, :], ones_full[:, :], ssq1[:, :], start=True, stop=True
            )
            g1_sb = pp.tile([1, 1], F32)
            csd1 = pp.tile([1, 20], F32, tag="csd1", name="csd1")
            nc.scalar.activation(
                out=csd1[:, :],
                in_=ps1[0:1, :],
                func=ACTF.Copy,
                accum_out=g1_sb[:, :],
            )
            nc.scalar.dma_start(out=cc_in.ap()[:, :], in_=g1_sb[:, :])
            nc.gpsimd.collective_compute(
                "AllReduce",
                ALU.add,
                replica_groups=[list(range(NCORES))],
                ins=[cc_in.ap().opt()],
                outs=[cc_out.ap().opt()],
            )

            # supertiles 14, 15 (AR2); last chunk is 512 KB so its square
            # (the AR2-input gate) finishes ~1.4 us after the last byte
            load_chunked(AR1_ST)
            sq_chunks(AR1_ST, ssq2, 0)
            amr_cols(AR1_ST, range(SUB))

            LASTSPLIT = ((0, 2), (2, 4), (4, 6), (6, 7), (7, 8))
            load_chunked(NST - 1, LASTSPLIT)
            sq_chunks(NST - 1, ssq2, 4, LASTSPLIT)
            amr_cols(NST - 1, range(SUB))

            # AR2 input: same PE+ACT path, ready ~2.5 us after last byte.
            ps2 = psmall.tile([P, 9], F32, tag="ps2")
            nc.tensor.matmul(
                ps2[:, :], ones_full[:, :], ssq2[:, :], start=True, stop=True
            )
            g2_sb = pp.tile([1, 1], F32)
            csd2 = pp.tile([1, 9], F32, tag="csd2", name="csd2")
            nc.scalar.activation(
                out=csd2[:, :],
                in_=ps2[0:1, :],
                func=ACTF.Copy,
                accum_out=g2_sb[:, :],
            )
            nc.scalar.dma_start(out=cc_in2.ap()[:, :], in_=g2_sb[:, :])
            nc.gpsimd.collective_compute(
                "AllReduce",
                ALU.add,
                replica_groups=[list(range(NCORES))],
                ins=[cc_in2.ap().opt()],
                outs=[cc_out2.ap().opt()],
            )

            # ---- pre-AR2 epilogue on the AR1-only scale ----
            # g1 arrives mid-stream; sqrt on ACT (default table), recip on
            # DVE. E1 = exp(scores * inv1) as soon as the last AMR lands.
            g1a_sb = pp.tile([1, 1], F32)
            nc.sync.dma_start(out=g1a_sb[:, :], in_=cc_out.ap()[:, :])
            g1b = psmall.tile([P, 1], F32, tag="g1b")
            nc.tensor.matmul(
                g1b[:, :], ones_row[:, :], g1a_sb[:, :], start=True, stop=True
            )
            # inv1 = rsqrt(g1): bit-trick seed + 2 Newton steps, DVE-only
            # (no ACT table loads on the tail path)
            g1r = pp.tile([P, 1], F32)
            nc.vector.tensor_copy(out=g1r[:, :], in_=g1b[:, :])
            magic = pp.tile([P, 1], I32)
            nc.vector.memset(magic[:, :], 0x5F3759DF)
            halfbits = pp.tile([P, 1], I32)
            nc.vector.tensor_scalar(
                out=halfbits[:, :], in0=g1r[:, :].bitcast(I32), scalar1=1,
                scalar2=None, op0=ALU.logical_shift_right,
            )
            y_i = pp.tile([P, 1], I32)
            nc.vector.scalar_tensor_tensor(
                out=y_i[:, :], in0=magic[:, :], scalar=1, in1=halfbits[:, :],
                op0=ALU.mult, op1=ALU.subtract,
            )
            y = y_i[:, :].bitcast(F32)
            ya = pp.tile([P, 1], F32)
            yb = pp.tile([P, 1], F32)
            inv1 = pp.tile([P, 1], F32)
            for it, (src_ap, dst) in enumerate(
                [(y, ya[:, :]), (ya[:, :], inv1[:, :])]
            ):
                gy2 = pp.tile([P, 1], F32, tag=f"gy2_{it}", name=f"gy2_{it}")
                nc.vector.scalar_tensor_tensor(
                    out=gy2[:, :], in0=src_ap, scalar=g1r[:, 0:1], in1=src_ap,
                    op0=ALU.mult, op1=ALU.mult,
                )
                nc.vector.tensor_scalar(
                    out=yb[:, :], in0=gy2[:, :], scalar1=-0.5, scalar2=1.5,
                    op0=ALU.mult, op1=ALU.add,
                )
                nc.vector.tensor_scalar(
                    out=dst, in0=src_ap, scalar1=yb[:, 0:1], scalar2=None,
                    op0=ALU.mult,
                )
            invg1 = pp.tile([P, 1], F32)  # 1/g1 = inv1^2, for r = g2/g1
            nc.vector.tensor_scalar(
                out=invg1[:, :], in0=inv1[:, :], scalar1=inv1[:, 0:1],
                scalar2=None, op0=ALU.mult,
            )

            e1 = pp.tile([P, NC_TILES], F32)
            nc.scalar.activation(
                out=e1[:, :], in_=scores[:, :], func=ACTF.Exp, scale=inv1[:, :]
            )
            w_t = pp.tile([P, NC_TILES], F32)
            nc.vector.tensor_scalar(
                out=w_t[:, :], in0=scores[:, :], scalar1=inv1[:, 0:1],
                scalar2=None, op0=ALU.mult,
            )
            em = pp.tile([P, NC_TILES], F32)
            nc.vector.tensor_tensor(
                out=em[:, :], in0=e1[:, :], in1=masks_all[:, :], op=ALU.mult
            )
            fm = pp.tile([P, NC_TILES], F32)
            nc.vector.tensor_tensor(
                out=fm[:, :], in0=em[:, :], in1=w_t[:, :], op=ALU.mult
            )
            zc = pp.tile([P, 2 * BPC], F32)  # [ze(4) | zf(4)] column sums
            nc.vector.tensor_reduce(
                out=zc[:, 0:BPC].rearrange("p (b o) -> p b o", o=1),
                in_=em[:, :].rearrange("p (b t) -> p b t", t=NT),
                axis=mybir.AxisListType.X,
                op=ALU.add,
            )
            nc.vector.tensor_reduce(
                out=zc[:, BPC : 2 * BPC].rearrange("p (b o) -> p b o", o=1),
                in_=fm[:, :].rearrange("p (b t) -> p b t", t=NT),
                axis=mybir.AxisListType.X,
                op=ALU.add,
            )
            zs = psmall.tile([P, 2 * BPC], F32, tag="zs")
            nc.tensor.matmul(
                zs[:, :], ones_full[:, :], zc[:, :], start=True, stop=True
            )
            zs_sb = pp.tile([P, 2 * BPC], F32)
            nc.vector.tensor_copy(out=zs_sb[:, :], in_=zs[:, :])

            # ---- post-AR2: linearized correction, ~10 tiny ops ----
            g2a_sb = pp.tile([1, 1], F32)
            nc.sync.dma_start(out=g2a_sb[:, :], in_=cc_out2.ap()[:, :])
            g2b = psmall.tile([P, 1], F32, tag="g2b")
            nc.tensor.matmul(
                g2b[:, :], ones_row[:, :], g2a_sb[:, :], start=True, stop=True
            )
            # r = g2/g1 ; delta = r*(-1/2 + r*(3/8 - (5/16) r))
            r_t = pp.tile([P, 1], F32)
            nc.vector.tensor_scalar(
                out=r_t[:, :], in0=g2b[:, :], scalar1=invg1[:, 0:1],
                scalar2=None, op0=ALU.mult,
            )
            h_t = pp.tile([P, 1], F32)  # h = 0.375*r - 0.5
            nc.vector.tensor_scalar(
                out=h_t[:, :], in0=r_t[:, :], scalar1=0.375, scalar2=-0.5,
                op0=ALU.mult, op1=ALU.add,
            )
            delta = pp.tile([P, 1], F32)  # delta = h * r
            nc.vector.tensor_scalar(
                out=delta[:, :], in0=h_t[:, :], scalar1=r_t[:, 0:1],
                scalar2=None, op0=ALU.mult,
            )
            # Z = ze + delta*zf ; invz = 1/Z
            z_t = pp.tile([P, BPC], F32)
            nc.vector.scalar_tensor_tensor(
                out=z_t[:, :],
                in0=zs_sb[:, BPC : 2 * BPC],
                scalar=delta[:, 0:1],
                in1=zs_sb[:, 0:BPC],
                op0=ALU.mult,
                op1=ALU.add,
            )
            invz = pp.tile([P, BPC], F32)
            nc.vector.reciprocal(out=invz[:, :], in_=z_t[:, :])
            # u = em + delta*fm ; o = u*invz_b + PERTURB
            u_t = pp.tile([P, NC_TILES], F32)
            nc.vector.scalar_tensor_tensor(
                out=u_t[:, :],
                in0=fm[:, :],
                scalar=delta[:, 0:1],
                in1=em[:, :],
                op0=ALU.mult,
                op1=ALU.add,
            )
            o_all = pp.tile([P, NC_TILES], F32)
            for b in range(BPC):
                nc.vector.tensor_scalar(
                    out=o_all[:, b * NT : (b + 1) * NT],
                    in0=u_t[:, b * NT : (b + 1) * NT],
                    scalar1=invz[:, b : b + 1],
                    scalar2=PERTURB,
                    op0=ALU.mult,
                    op1=ALU.add,
                )
            dst = out_ap[:, :, 0].rearrange("b (g p j) -> p b g j", p=P, j=SUB)
            src = o_all[:, :].rearrange("p (b g j) -> p b g j", b=BPC, j=SUB)
            nc.sync.dma_start(out=dst, in_=src)

    nc.compile()
    return nc


_NC_CACHE = None


def _get_nc():
    global _NC_CACHE
    if _NC_CACHE is None:
        _NC_CACHE = build()
    return _NC_CACHE


def make_in_maps(key, query, seq_lens):
    key = np.ascontiguousarray(np.asarray(key, dtype=np.float32))
    query = np.ascontiguousarray(np.asarray(query, dtype=np.float32))
    seq_lens = np.ascontiguousarray(np.asarray(seq_lens, dtype=np.int32))
    in_maps = []
    for c in range(NCORES):
        lo, hi = c * BPC, (c + 1) * BPC
        in_maps.append(
            {
                "key": key[lo:hi],
                "query": query[lo:hi],
                "seq_lens": seq_lens[lo:hi].reshape(1, BPC),
            }
        )
    return in_maps


def kernel(key, query, seq_lens, **run_kwargs):
    nc = _get_nc()
    in_maps = make_in_maps(key, query, seq_lens)
    res = run_bass_kernel_spmd(
        nc, in_maps, core_ids=list(range(NCORES)), **run_kwargs
    )
    outs = [res.results[c]["out"].reshape(BPC, S, 1) for c in range(NCORES)]
    full = np.concatenate(outs, axis=0).astype(np.float32)
    if run_kwargs:
        kernel.last_results = res  # expose profile info to test harness
    return full
